# revision 55
# baseline (speedup 1.0000x reference)
"""Multi-head self-attention Trainium2 kernel (B=8, S=1024, D=768, H=12, Hd=64).

Sharding: pure data-parallel, one batch element per NeuronCore (8 cores), no
collectives. Tuned against the TimelineSim cost model; hardware-validated.
Lineage: 193.1us (stub) -> 146.3/144.3us (fp16 pipeline) -> 137.1us (this).

Key structure (see git-less lineage in kernel_*.bak):
  * qkv is computed as THREE fp8e4 DoubleRow matmul passes
    (x_hi@w_hi + x_lo@w_hi + x_hi@w_lo), accumulating in one psum group.
    DoubleRow contracts 2 k-subtiles per instruction ([128, 2, N] operands)
    at 0.5 cycles per output column, so the 3-pass compensated-fp8 qkv
    costs 0.75x the fp16 version (82944 vs 110592 PE cycles) at ~2e-3
    final rel err. hi/lo split + per-tensor power-of-2 scales (SX=4 for x,
    SW=256 for w_qkv) keep e4m3 out of its subnormal range (w_qkv ~
    N(0, 1/768) would otherwise quantize to garbage). Descale is FREE:
    exp() absorbs (SX*SW)^-2 in its scale and the PV ones-column is SX*SW
    so the softmax denominator carries the numerator's scale factor.
  * All host prep is free: x is pre-transposed/quantized on host into the
    DR layout [p, hi/lo, half, dk2, j, 512]; w_qkv columns are permuted to
    [q0|k0|q1|k1|...|q5|k5|v] so the pair-0 critical load is one 512-col
    DMA that also covers pair 1 (w slices below 512 cols pay a 2x
    small-element DMA penalty - never slice narrower). b_qkv ships
    pre-scaled + pre-gathered [p, 12]; bv/bp ship pre-broadcast fp16.
  * ALL DMAs serialize on ONE DMA_ENGINES resource (~0.36ns/B-per-
    partition, x2 under 512B runs, +900ns completion sem). Load emission
    order here IS arrival order; it is deadline-ordered and the startup
    is DMA-serial-bound (~4us to first scores). The in-order PE must
    never meet a not-yet-loaded operand: pair-0's st2=1 qkT groups ride
    the fill (after x half 1 lands), not the pre-pair chains.
  * scores fp16 (contraction 64: DoubleRow would need fp8 q/k - accuracy
    says no); PV fp16 with exp stationary [128k, 128q] / v moving 65 cols
    (incl the ones column -> denominator); gpsimd normalize_recip.
    ScalarE exp = ~17.5us per head-pair window paces pairs 1-4.
  * proj staged: pjA1 (hp0+hp1 -> yA, +bias, DVE add) fills the
    otherwise-starved pairs 3-4; projA (hp2+hp3 + I@yA identity re-add)
    rides pair 5; tail = PV(5) + PE-transposes of outN(5) + hp4/hp5 +
    I@yA re-add, copies alternating ScalarE/DVE, stores on sync/scalar.
    outT for pairs 0-2,4 bounces via DRAM (DMA transpose); pairs 3/5
    transpose on the PE (pair-3 for pair-5's projA, pair-5 for latency).
  * PSUM (8 banks): scores 2x[128,1024] (4) + pv 2x[128,260] (2) +
    qkv/proj 2x[128,512] (2). expT 16 bufs (2 pairs) of [128,2048] fp16.
  * kernel() majority-votes over repeated runs: the remote pool
    intermittently returns one corrupted core (~1e-1 rel err on that
    batch); healthy runs are bit-deterministic.

Engine busy (sim): PE 118us, ScalarE 105us, DVE ~48us, Pool ~25us,
DMA ~34us; total 137.1us. Hardware rel err 1.915e-03 (gate 2e-2).
Failed directions (do not retry blindly): eager cross-pair proj staging
with merges round-robined across DVE/Pool (priority inversion: bulk adds
queued ahead of critical normalize/copies stall outN bounces by ~15us);
3-queue stores via gpsimd (SWDGE occupies Pool engine); store halving
(+8 DMA fixed costs); PV(5) chain pre-start straddling the pair-5/tail
boundary (holds both pv psum bufs, starves the tail quads).
"""
import numpy as np

B, S, D = 8, 1024, 768
H, Hd = 12, 64
D3 = 3 * D
N_CORES = 8
P = 128

# fp8 qkv: per-tensor power-of-2 scales (e4m3's normal range starts at
# 2^-6; unscaled w_qkv ~ N(0, 1/768) would land in subnormals and lose
# the hi/lo compensation). Descale is free: exp() absorbs (SX*SW)^-2 in
# its scale and the PV ones-column is SX*SW so the softmax denominator
# carries the same factor as the numerator.
SX = 4.0
SW = 256.0
VSC = SX * SW

_CACHE = {}
N_WARMUP = 24


def _build_nc():
    import concourse.bass as bass
    import concourse.mybir as mybir
    from concourse import bacc
    from concourse.tile import TileContext

    from concourse.masks import make_identity

    f32 = mybir.dt.float32
    fp16 = mybir.dt.float16
    fp8 = mybir.dt.float8e4
    AF = mybir.ActivationFunctionType
    DR = mybir.MatmulPerfMode.DoubleRow

    nc = bacc.Bacc("TRN2", target_bir_lowering=False, debug=False,
                   num_devices=N_CORES)

    # xT (host-pre-transposed, fp8): [p, hi/lo, half, dk2, j, seq-in-half],
    # d = (dk2*2 + j)*128 + p. hi is needed ~2us before lo, so each
    # (hi/lo, half) is its own 3KB-contiguous DMA.
    x_d = nc.declare_dram_parameter("x8", [P, 2, 2, 3, 2, 512], fp8,
                                    isOutput=False)
    # w_qkv fp8 in DR stationary layout [p, hi/lo, dk2, j, col], columns
    # host-permuted to [q0|k0|q1|k1|...|q5|k5|v]
    w_d = nc.declare_dram_parameter("w8", [P, 2, 3, 2, D3], fp8,
                                    isOutput=False)
    # q/k bias columns pre-gathered [p, 12] (contiguous rows, no strided DMA)
    bqk_d = nc.declare_dram_parameter("bqk_p", [P, 12], f32, isOutput=False)
    # pre-broadcast bv/bp rows (fp16): [p, 2, 768]
    b2_d = nc.declare_dram_parameter("b2", [P, 2, D], fp16, isOutput=False)
    wproj_d = nc.declare_dram_parameter("w_proj", [D, D], fp16, isOutput=False)
    out_d = nc.declare_dram_parameter("out", [S, D], fp16, isOutput=True)

    KD = D // P            # 6 k-chunks of 128 over D
    ST = S // P            # 8 s-tiles of 128
    NPAIR = H // 2         # 6 head pairs

    with TileContext(nc) as tc:
        with tc.tile_pool(name="consts", bufs=1) as consts, \
             tc.tile_pool(name="big", bufs=1) as big, \
             tc.tile_pool(name="work", bufs=1) as work, \
             tc.tile_pool(name="ypool", bufs=3) as ypool, \
             tc.tile_pool(name="dpool", bufs=1, space="DRAM") as dpool, \
             tc.tile_pool(name="ps", bufs=1, space="PSUM") as ps:

            identf = consts.tile([P, P], fp16)
            # ------------- loads. ALL DMAs serialize on one DMA-engine
            # resource (~0.36ns/B-per-partition, x2 under 512B runs, +900ns
            # completion sem), so emission order here IS the arrival order.
            # Strict deadline order: pair-0 stationaries/movings (hi before
            # lo), x half 1, v weights, pairs 1-2 qk, biases, the rest.
            xsb = big.tile([P, 2, 2, 3, 2, 512], fp8, name="x8")
            wsb = big.tile([P, 2, 3, 2, D3], fp8, name="w8")
            wp_big = big.tile([P, KD, D], fp16, name="wproj")
            wp_sb = [wp_big[:, kd] for kd in range(KD)]

            def x_mov(hl, i, c0, c1):
                """moving DR operand [128, 2, c1-c0] for dk2 chunk i"""
                h = c0 // 512
                assert c1 <= (h + 1) * 512
                return xsb[:, hl, h, i, :, c0 - h * 512:c1 - h * 512]

            def w_sl(hl, i, c0, c1):
                return wsb[:, hl, i, :, c0:c1]

            def w_load(eng, hl, c0, c1):
                eng.dma_start(out=wsb[:, hl, :, :, c0:c1],
                              in_=w_d[:, hl, :, :, c0:c1])

            nc.gpsimd.memset(identf[:], 0.0)
            w_load(nc.sync, 0, 0, 512)      # q0k0+q1k1 hi (512-col: no 2x)
            nc.gpsimd.dma_start(out=xsb[:, 0, 0], in_=x_d[:, 0, 0])
            nc.scalar.dma_start(out=xsb[:, 1, 0], in_=x_d[:, 1, 0])
            w_load(nc.sync, 1, 0, 512)      # q0k0+q1k1 lo
            bqk_cols = consts.tile([P, 12], f32)
            nc.gpsimd.dma_start(out=bqk_cols[:], in_=bqk_d[:])
            w_load(nc.scalar, 0, 2 * D, D3)  # v hi
            w_load(nc.sync, 1, 2 * D, D3)    # v lo
            nc.gpsimd.dma_start(out=xsb[:, 0, 1], in_=x_d[:, 0, 1])
            nc.scalar.dma_start(out=xsb[:, 1, 1], in_=x_d[:, 1, 1])
            w_load(nc.sync, 0, 512, 1536)    # pairs 2-5 hi
            b2 = consts.tile([P, 2, D], fp16)
            nc.gpsimd.dma_start(out=b2[:], in_=b2_d[:])
            w_load(nc.scalar, 1, 512, 1536)  # pairs 2-5 lo
            nc.sync.dma_start(
                out=wp_big[:],
                in_=wproj_d[:].rearrange("(kd p) n -> p kd n", p=P))
            bv_bc = b2[:, 0]
            bp_bc = b2[:, 1]

            qkT = [big.tile([P, S], fp16, name=f"qkT{mt}") for mt in range(12)]
            v_sb = [big.tile([P, 65 * H], fp16, name=f"v{st}") for st in range(ST)]
            # outT[p]: [128 d (2 heads), 1024 q], from the DMA transpose
            outT = [big.tile([P, S], fp16, name=f"outT{p_i}") for p_i in range(NPAIR)]
            # normalized attention out, [q, d] layout, per pair: [128, 8 sq, 128]
            # (rotating: only the pair being normalized + the one being stored
            # are alive)
            outN = {}

            def outN_tile(p_i):
                if p_i not in outN:
                    outN[p_i] = work.tile([P, ST, P], fp16, tag="outN", bufs=2,
                                          name=f"outN{p_i}")
                return outN[p_i]
            outN_d = [dpool.tile([S, P], fp16, name=f"outNd{p_i}")
                      for p_i in range(NPAIR - 1)]
            # early proj partial (head-pairs 0..3), fp16 is plenty here
            yA = [big.tile([P, D], fp16, name=f"yA{st}") for st in range(ST)]

            # qkv = x@w as 3 fp8 DoubleRow passes (hi@hi + lo@hi + hi@lo),
            # all accumulating in one psum group; result scaled by SX*SW.
            QKV_PASSES = ((0, 0), (0, 1), (1, 0))  # (w hi/lo, x hi/lo)

            def wcol(mt):
                """column of head mt in the permuted [q0|k0|q1|k1|...] layout"""
                return 256 * mt if mt < 6 else 256 * (mt - 6) + 128

            def emit_qkT_group(mt, st2):
                pq = ps.tile([P, 512], f32, tag="qkv", bufs=2,
                             name=f"pq{mt}_{st2}")
                w0 = wcol(mt)
                bj = 2 * mt if mt < 6 else 2 * (mt - 6) + 1
                for cc in range(2):
                    c0 = st2 * 512 + cc * 256
                    n9 = 0
                    for whl, xhl in QKV_PASSES:
                        for i in range(3):
                            nc.tensor.matmul(
                                pq[:, cc * 256:(cc + 1) * 256],
                                w_sl(whl, i, w0, w0 + P),
                                x_mov(xhl, i, c0, c0 + 256),
                                start=(n9 == 0), stop=(n9 == 8),
                                perf_mode=DR)
                            n9 += 1
                nc.vector.tensor_scalar_add(
                    qkT[mt][:, st2 * 512:(st2 + 1) * 512], pq[:],
                    bqk_cols[:, bj:bj + 1])

            def emit_v_group(st, n0):
                nw, h0 = (512, 0) if n0 == 0 else (256, 8)
                pv = ps.tile([P, 512], f32, tag="qkv", bufs=2,
                             name=f"pvv{st}_{n0}")
                for cc in range(nw // 256):
                    w0 = 2 * D + n0 + cc * 256
                    n9 = 0
                    for whl, xhl in QKV_PASSES:
                        for i in range(3):
                            nc.tensor.matmul(
                                pv[:, cc * 256:cc * 256 + 256],
                                x_mov(xhl, i, st * P, (st + 1) * P),
                                w_sl(whl, i, w0, w0 + 256),
                                start=(n9 == 0), stop=(n9 == 8),
                                perf_mode=DR)
                            n9 += 1
                nh = nw // Hd
                nc.vector.tensor_add(
                    v_sb[st][:, 65 * h0:65 * h0 + 65 * nh]
                    .rearrange("p (h c) -> p h c", c=65)[:, :, 0:Hd],
                    pv[:, 0:nw].rearrange("p (h c) -> p h c", c=Hd),
                    bv_bc[:, n0:n0 + nw].rearrange("p (h c) -> p h c", c=Hd))

            # ------------- PV for one (pair, 2 adjacent sq): ----------------
            # exp stationary [128k, 128q], v moving [128k, 65]; out [q, 65]
            # accumulated over the 8 k-tiles.  4 accumulation groups live in
            # one [128, 260] psum tile: (sq0 h0)(sq0 h1)(sq1 h0)(sq1 h1).
            pv_open = {}

            def emit_pv_start(p_i, sq2, expT, sk_hi):
                """open a PV accumulation chain over sk < sk_hi (no stop);
                lets most of the last pair's PV run before its final exp"""
                po = ps.tile([P, 260], f32, tag="pv", bufs=2,
                             name=f"po{p_i}_{sq2}")
                pv_open[(p_i, sq2)] = po
                for sql in range(2):
                    sq = 2 * sq2 + sql
                    for hh in range(2):
                        o0 = 130 * sql + 65 * hh
                        for sk in range(sk_hi):
                            nc.tensor.matmul(
                                po[:, o0:o0 + 65],
                                expT[sk][:, hh * 1024 + sq * P:hh * 1024 + (sq + 1) * P],
                                v_sb[sk][:, 65 * (2 * p_i + hh):65 * (2 * p_i + hh) + 65],
                                start=(sk == 0), stop=False,
                                skip_group_check=True)

            def emit_pv_finish(p_i, sq2, expT, sk_lo):
                po = pv_open.pop((p_i, sq2))
                for sql in range(2):
                    sq = 2 * sq2 + sql
                    for hh in range(2):
                        o0 = 130 * sql + 65 * hh
                        for sk in range(sk_lo, ST):
                            nc.tensor.matmul(
                                po[:, o0:o0 + 65],
                                expT[sk][:, hh * 1024 + sq * P:hh * 1024 + (sq + 1) * P],
                                v_sb[sk][:, 65 * (2 * p_i + hh):65 * (2 * p_i + hh) + 65],
                                start=False, stop=(sk == ST - 1),
                                skip_group_check=True)
                po_sb = work.tile([P, 260], f32, tag="posb", bufs=3,
                                  name=f"posb{p_i}_{sq2}")
                nc.vector.tensor_copy(po_sb[:], po[:])
                for sql in range(2):
                    sq = 2 * sq2 + sql
                    for hh in range(2):
                        o0 = 130 * sql + 65 * hh
                        nc.gpsimd.normalize_recip(
                            outN_tile(p_i)[:, sq, hh * Hd:(hh + 1) * Hd],
                            po_sb[:, o0:o0 + Hd],
                            po_sb[:, o0 + Hd:o0 + Hd + 1])

            def emit_pv_sqpair(p_i, sq2, expT):
                emit_pv_start(p_i, sq2, expT, ST)
                emit_pv_finish(p_i, sq2, expT, ST)

            def emit_outT_quad(p_i, half):
                """PE-transpose 4 sq tiles of outN[p] into outT[p] [d, q].
                Used for the last pair only (latency); earlier pairs bounce
                through DRAM and the DMA xbar, off the critical PE."""
                tp = ps.tile([P, 512], fp16, tag="pv", bufs=2,
                             name=f"tp{p_i}_{half}")
                for sql in range(4):
                    sq = 4 * half + sql
                    nc.tensor.transpose(tp[:, sql * P:(sql + 1) * P],
                                        outN_tile(p_i)[:, sq, :], identf[:])
                nc.vector.tensor_copy(
                    outT[p_i][:, half * 512:(half + 1) * 512], tp[:])

            def emit_outT_dma(p_i):
                nc.sync.dma_start(
                    out=outN_d[p_i][:].rearrange("(s q) d -> q s d", q=P),
                    in_=outN_tile(p_i)[:])
                nc.sync.dma_start(out=outT[p_i][:], in_=outN_d[p_i][:],
                                  transpose=True)

            # ------------- proj: staged. pjA1 (pairs 3-4): hp0+hp1 -> yA
            # (+bias, DVE). projA2 (pair 5): hp2+hp3 + I@yA readd -> yA.
            # Tail: hp4+hp5 + I@yA readd.
            def emit_pjA1(st, n0):
                nw = 512 if n0 == 0 else 256
                py = ps.tile([P, 512], f32, tag="qkv", bufs=2,
                             name=f"pjA1_{st}_{n0}")
                for k in (0, 1):
                    nc.tensor.matmul(
                        py[:, 0:nw],
                        outT[k][:, st * P:(st + 1) * P],
                        wp_sb[k][:, n0:n0 + nw],
                        start=(k == 0), stop=(k == 1))
                nc.vector.tensor_add(yA[st][:, n0:n0 + nw], py[:, 0:nw],
                                     bp_bc[:, n0:n0 + nw])

            def emit_projA(st, n0):
                nw = 512 if n0 == 0 else 256
                py = ps.tile([P, 512], f32, tag="qkv", bufs=2,
                             name=f"pyA{st}_{n0}")
                for k in (2, 3):
                    nc.tensor.matmul(
                        py[:, 0:nw],
                        outT[k][:, st * P:(st + 1) * P],
                        wp_sb[k][:, n0:n0 + nw],
                        start=(k == 2), stop=False)
                nc.tensor.matmul(py[:, 0:nw], identf[:],
                                 yA[st][:, n0:n0 + nw],
                                 start=False, stop=True)
                nc.vector.tensor_copy(yA[st][:, n0:n0 + nw], py[:, 0:nw])

            ytiles = {}

            def emit_proj_tail(st, n0):
                nw = 512 if n0 == 0 else 256
                tag = "scores" if (st + (n0 != 0)) % 2 == 0 else "qkv"
                py = ps.tile([P, 512], f32, tag=tag, bufs=2, name=f"pyB{st}_{n0}")
                for k in (4, 5):
                    nc.tensor.matmul(
                        py[:, 0:nw],
                        outT[k][:, st * P:(st + 1) * P],
                        wp_sb[k][:, n0:n0 + nw],
                        start=(k == 4), stop=False)
                nc.tensor.matmul(py[:, 0:nw], identf[:],
                                 yA[st][:, n0:n0 + nw],
                                 start=False, stop=True)
                if st not in ytiles:
                    ytiles[st] = ypool.tile([P, D], fp16, tag="y", bufs=8,
                                            name=f"y{st}")
                yt = ytiles[st]
                if (st + (n0 != 0)) % 2 == 1:
                    nc.scalar.copy(yt[:, n0:n0 + nw], py[:, 0:nw])
                else:
                    nc.vector.tensor_copy(yt[:, n0:n0 + nw], py[:, 0:nw])
                if n0 != 0:
                    eng = nc.sync if st % 2 == 0 else nc.scalar
                    eng.dma_start(out=out_d[st * P:(st + 1) * P, :], in_=yt[:])

            # ------------- pair loop: scores + exp, fill interleaved --------
            def emit_pair(p_i, fill, front=None, expT=None):
                """fill: list of (closure, pe_ns); consumed evenly (by count)
                across sk steps 0..6 (after each sk's scores+exp). expT (if
                given) is filled in place so fill closures can reference it."""
                front = front or []
                qt, kt = qkT[p_i], qkT[6 + p_i]
                expT = [] if expT is None else expT
                for sk in range(ST):
                    et = work.tile([P, 2048], fp16, tag="expT", bufs=16,
                                   name=f"expT{p_i}_{sk}")
                    for hh in range(2):
                        lo, hi = hh * Hd, (hh + 1) * Hd
                        pscore = ps.tile([P, 1024], f32, tag="scores", bufs=2,
                                         name=f"psc{p_i}_{sk}_{hh}")
                        for sq2 in range(2):
                            nc.tensor.matmul(
                                pscore[:, sq2 * 512:(sq2 + 1) * 512],
                                kt[lo:hi, sk * P:(sk + 1) * P],
                                qt[lo:hi, sq2 * 512:(sq2 + 1) * 512],
                                start=True, stop=True)
                        nc.scalar.activation(et[:, hh * 1024:(hh + 1) * 1024],
                                             pscore[:], AF.Exp,
                                             scale=float(Hd) ** -0.5 / VSC ** 2)
                    expT.append(et)
                    if sk < len(front):
                        for g, _ in front[sk]:
                            g()
                    a0 = (sk * len(fill)) // ST
                    a1 = ((sk + 1) * len(fill)) // ST
                    for g, _ in fill[min(a0, len(fill)):min(a1, len(fill))]:
                        g()
                return expT

            # ---------------- schedule ----------------
            # ones column = VSC so the softmax denominator carries the same
            # SX*SW factor as the scaled v numerator (ratio descales free)
            for st in range(ST):
                nc.gpsimd.memset(
                    v_sb[st][:].rearrange("p (h c) -> p h c", c=65)[:, :, 64:65],
                    VSC)

            # PE warmup: the pstate model runs cold-start matmuls at up to
            # 3.7x slow cycles until 3us of continuous busy. Keep the PE
            # spinning on identity transposes while the loads land so the
            # first real matmuls issue at full speed.
            for w in range(N_WARMUP):
                wtp = ps.tile([P, P], fp16, tag="pv", bufs=2, name=f"wu{w}")
                nc.tensor.transpose(wtp[:], identf[:], identf[:])
            # the real identity (for pair-4/5 transposes) is built only now:
            # its writes wait on the warmup reads of the zeroed tile above
            make_identity(nc, identf[:])

            # pair 0 needs qkT 0 (q) and 6 (k) for sk/sq of x half 0 only
            # at first: the st2=1 groups ride pair-0's fill once x half 1
            # lands (~10us), keeping the in-order PE off not-ready loads.
            for mt in (0, 6):
                emit_qkT_group(mt, 0)

            def qg(mt, st2):
                return (lambda: emit_qkT_group(mt, st2), 960)

            def vg(st, n0):
                return (lambda: emit_v_group(st, n0),
                        960 if n0 == 0 else 480)

            def pvg(p_i, sq2, expT):
                return (lambda: emit_pv_sqpair(p_i, sq2, expT), 866)

            def pjS(st, n0, ks, first=False, eng=None):
                return (lambda: emit_proj_stage(st, n0, ks, first,
                                                eng or nc.vector),
                        len(ks) * (213 if n0 == 0 else 107))

            def quad(p_i, half):
                return (lambda: emit_outT_quad(p_i, half), 213)

            def tdma(p_i):
                return (lambda: emit_outT_dma(p_i), 10)

            # ---- schedule: baseline structure. PV of the previous pair
            # at the front of each pair, qkT JIT, v inside pair 0, proj
            # (hp0-3) inside pair 5, tail does PV(5) + hp4/5 + readd.
            def projA(st, n0):
                return (lambda: emit_projA(st, n0), 640 if n0 == 0 else 320)

            def pjA1(st, n0):
                return (lambda: emit_pjA1(st, n0), 427 if n0 == 0 else 213)

            expT_prev = None
            for p_i in range(NPAIR):
                fill = []
                if expT_prev is not None:
                    fill += [pvg(p_i - 1, sq2, expT_prev) for sq2 in range(4)]
                    if p_i == 4:
                        fill += [qg(p_i + 1, 0),
                                 quad(3, 0),
                                 qg(p_i + 1, 1),
                                 quad(3, 1),
                                 qg(7 + p_i, 0), qg(7 + p_i, 1)]
                        fill += [pjA1(st, n0)
                                 for st in range(6, ST) for n0 in (0, 512)]
                        expT_prev = emit_pair(p_i, fill)
                        continue
                    else:
                        fill.append(tdma(p_i - 1))
                if p_i == 0:
                    fill += [qg(1, 0), qg(1, 1), qg(7, 0), qg(7, 1)]
                    fill += [vg(0, 0), vg(0, 512), vg(1, 0), vg(1, 512),
                             qg(0, 1), vg(2, 0), vg(2, 512), qg(6, 1)]
                    fill += [vg(st, n0) for st in range(3, ST)
                             for n0 in (0, 512)]
                elif p_i < NPAIR - 1 and p_i != 4:
                    fill += [qg(p_i + 1, 0), qg(p_i + 1, 1),
                             qg(7 + p_i, 0), qg(7 + p_i, 1)]
                    if p_i == 3:
                        fill += [pjA1(st, n0)
                                 for st in range(6) for n0 in (0, 512)]
                if p_i == 5:
                    projs = [projA(st, n0) for st in range(ST)
                             for n0 in (0, 512)]
                    base = list(fill)
                    fill = []
                    pi2 = 0
                    for g in base:
                        fill.append(g)
                        fill += projs[pi2:pi2 + 2]
                        pi2 += 2
                    fill += projs[pi2:]
                expT_prev = emit_pair(p_i, fill)

            # tail: finish PV(5) 0/1 (only sk7 gates on the last exp),
            # first outN half + hp4/5 proj for st 0-3 while PV(5) 2/3 run,
            # then the second half.
            for sq2 in range(4):
                emit_pv_sqpair(5, sq2, expT_prev)
            emit_outT_quad(5, 0)
            for st in range(4):
                for n0 in (0, 512):
                    emit_proj_tail(st, n0)
            emit_outT_quad(5, 1)
            for st in range(4, ST):
                for n0 in (0, 512):
                    emit_proj_tail(st, n0)

    nc.finalize()
    return nc


def _get_runner():
    """Build + compile once; return a callable(list_of_in_maps) -> out dicts."""
    if "runner" in _CACHE:
        return _CACHE["runner"]

    import jax
    from jax.sharding import Mesh, PartitionSpec
    from jax.experimental.shard_map import shard_map
    import concourse.mybir as mybir
    from concourse.bass2jax import (_bass_exec_p, install_neuronx_cc_hook,
                                    partition_id_tensor)

    nc = _build_nc()
    install_neuronx_cc_hook()

    in_names = []
    out_names = []
    out_avals = []
    zero_out_shapes = []
    partition_name = nc.partition_id_tensor.name if nc.partition_id_tensor else None
    for alloc in nc.m.functions[0].allocations:
        if not isinstance(alloc, mybir.MemoryLocationSet):
            continue
        name = alloc.memorylocations[0].name
        if alloc.kind == "ExternalInput":
            if name != partition_name:
                in_names.append(name)
        elif alloc.kind == "ExternalOutput":
            out_names.append(name)
            shape = tuple(alloc.tensor_shape)
            dtype = mybir.dt.np(alloc.dtype)
            out_avals.append(jax.core.ShapedArray(shape, dtype))
            zero_out_shapes.append((shape, dtype))

    n_params = len(in_names)
    n_outs = len(out_avals)
    all_in_names = list(in_names) + list(out_names)
    if partition_name is not None:
        all_in_names.append(partition_name)
    donate = tuple(range(n_params, n_params + n_outs))

    def _body(*args):
        operands = list(args)
        if partition_name is not None:
            operands.append(partition_id_tensor())
        outs = _bass_exec_p.bind(
            *operands,
            out_avals=tuple(out_avals),
            in_names=tuple(all_in_names),
            out_names=tuple(out_names),
            lowering_input_output_aliases=(),
            sim_require_finite=True,
            sim_require_nnan=True,
            nc=nc,
        )
        return tuple(outs)

    devices = jax.devices()[:N_CORES]
    mesh = Mesh(np.asarray(devices), ("core",))
    in_specs = (PartitionSpec("core"),) * (n_params + n_outs)
    out_specs = (PartitionSpec("core"),) * n_outs
    sharded = jax.jit(
        shard_map(_body, mesh=mesh, in_specs=in_specs, out_specs=out_specs,
                  check_rep=False),
        donate_argnums=donate, keep_unused=True)

    def runner(in_maps):
        concat_in = [
            np.concatenate([np.asarray(in_maps[c][nm]) for c in range(N_CORES)],
                           axis=0)
            for nm in in_names
        ]
        concat_zeros = [
            np.zeros((N_CORES * sh[0], *sh[1:]), dt) for sh, dt in zero_out_shapes
        ]
        out_arrs = sharded(*concat_in, *concat_zeros)
        out_arrs = [np.asarray(a) for a in out_arrs]
        return [
            {nm: out_arrs[i].reshape(N_CORES, *out_avals[i].shape)[c]
             for i, nm in enumerate(out_names)}
            for c in range(N_CORES)
        ]

    _CACHE["runner"] = runner
    return runner


def _split8(a):
    """fp8e4 hi/lo pair of a (already scaled); returns (hi, lo) e4m3."""
    import ml_dtypes
    E4 = ml_dtypes.float8_e4m3
    hi = a.astype(E4)
    lo = (a - hi.astype(np.float32)).astype(E4)
    return hi, lo


def kernel(x, w_qkv, b_qkv, w_proj, b_proj):
    x = np.asarray(x, dtype=np.float32)
    w_qkv = np.asarray(w_qkv, dtype=np.float32)
    b_qkv = np.asarray(b_qkv, dtype=np.float32)
    w_proj = np.asarray(w_proj, dtype=np.float32)
    b_proj = np.asarray(b_proj, dtype=np.float32)

    # xT fp8 in DoubleRow layout [p, hi/lo, half, dk2, j, 512]
    xT = x.transpose(0, 2, 1)                     # [B, D, S]
    xT = xT.reshape(B, 3, 2, P, 2, 512)           # [B, dk2, j, p, half, s]
    xT = xT.transpose(0, 3, 4, 1, 2, 5)           # [B, p, half, dk2, j, s]
    x_hi, x_lo = _split8(xT * SX)
    x8 = np.ascontiguousarray(np.stack([x_hi, x_lo], axis=1))

    # permute w_qkv/b_qkv columns to [q0|k0|q1|k1|...|q5|k5|v] so the
    # device's critical loads are few wide DMAs
    perm = np.concatenate([np.r_[128 * p:128 * p + 128,
                                 D + 128 * p:D + 128 * p + 128]
                           for p in range(6)] + [np.r_[2 * D:D3]])
    w_qkv = w_qkv[:, perm]
    b_qkv = b_qkv[perm]

    # w fp8 packed [p, hi/lo, dk2, j, col]
    wq = w_qkv.reshape(3, 2, P, D3)               # [dk2, j, p, col]
    wq = wq.transpose(2, 0, 1, 3)                 # [p, dk2, j, col]
    w_hi, w_lo = _split8(wq * SW)
    w8 = np.ascontiguousarray(np.stack([w_hi, w_lo], axis=1))

    b_qkv_s = b_qkv * VSC                         # descaled via exp/ones-col
    # q/k bias pre-gathered [p, 12] (cols j: [q0,k0,q1,k1,...])
    bqk_p = np.ascontiguousarray(b_qkv_s[0:12 * P].reshape(12, P).T)
    # pre-broadcast bv (scaled) and bp rows, fp16: [p, 2, 768]
    b2 = np.ascontiguousarray(np.broadcast_to(
        np.stack([b_qkv_s[2 * D:], b_proj]).astype(np.float16)[None],
        (P, 2, D)))
    w_proj16 = np.ascontiguousarray(w_proj.astype(np.float16))

    runner = _get_runner()
    in_maps = [
        {"x8": x8[c], "w8": w8, "bqk_p": bqk_p, "b2": b2,
         "w_proj": w_proj16}
        for c in range(N_CORES)
    ]

    def run_once():
        outs = runner(in_maps)
        return np.stack([outs[c]["out"] for c in range(N_CORES)], axis=0)

    # The remote pool occasionally returns a corrupted result from one
    # core (~1e-1 rel err on that batch, intermittent). Healthy runs are
    # bit-deterministic, so majority-vote per core across repeat runs.
    r1 = run_once()
    r2 = run_once()
    if not np.array_equal(r1, r2):
        per_core = [[r1[c], r2[c]] for c in range(N_CORES)]
        out = np.empty_like(r1)
        for _ in range(3):
            agree = [any(np.array_equal(a, b)
                         for i, a in enumerate(votes) for b in votes[i + 1:])
                     for votes in per_core]
            if all(agree):
                break
            r3 = run_once()
            for c in range(N_CORES):
                per_core[c].append(r3[c])
        for c in range(N_CORES):
            votes = per_core[c]
            pick = votes[-1]
            for i, a in enumerate(votes):
                if any(np.array_equal(a, b) for b in votes[i + 1:]):
                    pick = a
                    break
            out[c] = pick
        r1 = out
    return r1.astype(np.float32)

